# revision 1
# baseline (speedup 1.0000x reference)
"""Deformable head attention kernel for 8 Trainium2 NeuronCores.

Sharding: core i handles batch b = i//2 and head-group hg = i%2 (heads
4*hg..4*hg+3, all 4096 queries). The reference's final reshape maps output
pixel p' to head p'//512's features of queries 8t..8t+7 (t = p' % 512), so a
head-group owns output rows [hg*32, hg*32+32) exactly -- fully local per core.

Per core:
  - channel-major matmuls on PE (q/offset/attention/key projections, output)
  - softmax + bilinear coordinate/weight math on DVE/ACT in [q-partition] layout
  - per-(head,scale) quad maps (2x2x32ch tokens, 256B bf16) in DRAM
  - SWDGE dma_gather fetches one quad per sample; DVE does the weighted reduce
  - output scramble via a DRAM round-trip, then the Wm projection
"""
import os
import numpy as np
from contextlib import ExitStack

import concourse.bass as bass
import concourse.tile as tile
from concourse import bacc, mybir
from concourse.bass_utils import run_bass_kernel_spmd
from concourse.masks import make_identity

F32 = mybir.dt.float32
I32 = mybir.dt.int32
I16 = mybir.dt.int16
BF16 = mybir.dt.bfloat16
OP = mybir.AluOpType
AF = mybir.ActivationFunctionType

HEADS, KPTS, SCALES, D = 8, 4, 4, 256
DK = D // HEADS              # 32
HL = 4                       # heads per core
B, H, W = 4, 64, 64
Q = 4096                     # queries per core (full image)
QC = Q // 128                # 32 q-chunks
HW_SIZES = [(16, 16), (32, 32), (64, 64), (128, 128)]
POS = [h * w for h, w in HW_SIZES]
TCAP = [p + w + 4 for p, (h, w) in zip(POS, HW_SIZES)]
NCORES = 8
NIDX = KPTS * 2048           # 8192 gather indices per (head, scale, q-half)

_cache = {}
PH = os.environ.get("KPH", "IDEF")


def _build():
    nc = bacc.Bacc("TRN2", target_bir_lowering=False, debug=False)

    d_queryT = nc.dram_tensor("queryT", [2, 128, Q], F32, kind="ExternalInput")
    d_keysT = [nc.dram_tensor(f"keysT{l}", [2, 128, POS[l]], F32, kind="ExternalInput")
               for l in range(SCALES)]
    d_refx = nc.dram_tensor("refx", [128, QC], F32, kind="ExternalInput")
    d_refy = nc.dram_tensor("refy", [128, QC], F32, kind="ExternalInput")
    d_Wq = nc.dram_tensor("Wq", [2, 128, D], F32, kind="ExternalInput")
    d_Wk = nc.dram_tensor("Wk", [2, 128, 128], F32, kind="ExternalInput")
    d_Woff = nc.dram_tensor("WoffP", [2, 128, 128], F32, kind="ExternalInput")
    d_WA = nc.dram_tensor("WA", [2, 128, 64], F32, kind="ExternalInput")
    d_Wm = nc.dram_tensor("Wm", [2, 128, D], F32, kind="ExternalInput")
    d_bq = nc.dram_tensor("bq", [2, 128, 1], F32, kind="ExternalInput")
    d_bk = nc.dram_tensor("bk", [128, 1], F32, kind="ExternalInput")
    d_bm = nc.dram_tensor("bm", [2, 128, 1], F32, kind="ExternalInput")
    d_boff = nc.dram_tensor("boffE", [128, 128], F32, kind="ExternalInput")
    d_bA = nc.dram_tensor("bAE", [128, 64], F32, kind="ExternalInput")
    cnames = ["wl_t", "wlm1_t", "wlm2_t", "wlp1_t", "hlm1_t", "hlm2_t",
              "hl_t", "cofx_t", "cofy_t"]
    d_const = {n: nc.dram_tensor(n, [128, 64], F32, kind="ExternalInput")
               for n in cnames}

    d_out = nc.dram_tensor("outT", [2, 128, 2048], F32, kind="ExternalOutput")
    d_map = [nc.dram_tensor(f"map{l}", [HL, TCAP[l], 128], BF16)
             for l in range(SCALES)]
    d_featD = nc.dram_tensor("featD", [2048, 256], F32)   # scrambled [o, c']

    with tile.TileContext(nc) as tc, ExitStack() as ctx:
        wpool = ctx.enter_context(tc.tile_pool(name="weights", bufs=1))
        ppool = ctx.enter_context(tc.tile_pool(name="persist", bufs=1))
        psum = ctx.enter_context(tc.tile_pool(name="psum", bufs=2, space="PSUM"))

        def load2(d, n, nm):
            t = [wpool.tile([128, n], F32, tag=f"{nm}{i}", name=f"{nm}{i}")
                 for i in range(2)]
            for i in range(2):
                nc.sync.dma_start(t[i][:], d[i])
            return t

        def load1(d, shape, nm):
            t = wpool.tile(shape, F32, tag=nm, name=nm)
            nc.sync.dma_start(t[:], d[:])
            return t

        Wq = load2(d_Wq, D, "Wq"); Wk = load2(d_Wk, 128, "Wk")
        Woff = load2(d_Woff, 128, "Woff"); WA = load2(d_WA, 64, "WA")
        Wm = load2(d_Wm, D, "Wm")
        bq = load2(d_bq, 1, "bq"); bm = load2(d_bm, 1, "bm")
        bk = load1(d_bk, [128, 1], "bk")
        boffE = load1(d_boff, [128, 128], "boffE")
        bAE = load1(d_bA, [128, 64], "bAE")
        C = {n: load1(d_const[n], [128, 64], n) for n in cnames}
        refx = load1(d_refx, [128, QC], "refx")
        refy = load1(d_refy, [128, QC], "refy")
        ident = wpool.tile([128, 128], F32, tag="ident", name="ident")
        make_identity(nc, ident[:])
        tok16 = wpool.tile([128, QC, 64], I16, tag="tok16", name="tok16")

        def bh(t, n=64):
            return t[:].rearrange("p (o f) -> p o f", o=1).broadcast_to([128, QC, n])

        W4 = ppool.tile([128, HL, SCALES, KPTS, QC, 4, 2], BF16, tag="W4", name="W4")
        feat = ppool.tile([128, QC, HL, DK], F32, tag="feat", name="feat")

        # zero map edge regions first (independent of everything else)
        with tc.tile_pool(name="zp", bufs=1) as zp:
            zt = zp.tile([128, 288], BF16, tag="zero", name="zero")
            nc.vector.memset(zt[:], 0)
            for l in range(SCALES):
                _, wl = HW_SIZES[l]
                dmv = d_map[l][:].rearrange("h t e -> h (t e)")
                ze1 = wl + 4
                ze2 = TCAP[l] - POS[l] + wl
                for h in range(HL):
                    nc.sync.dma_start(
                        dmv[h, 0:ze1 * 128].rearrange("(p f) -> p f", p=128),
                        zt[:, 0:ze1])
                    nc.sync.dma_start(
                        dmv[h, (POS[l] - wl) * 128:TCAP[l] * 128]
                            .rearrange("(p f) -> p f", p=128),
                        zt[:, 0:ze2])
        tc.strict_bb_all_engine_barrier()

        with tc.tile_pool(name="pbc", bufs=1) as pbc, \
             tc.tile_pool(name="poff", bufs=1) as poff:
            Aw = pbc.tile([128, QC, 64], F32, tag="Aw", name="Aw")
            offx = poff.tile([128, QC, 64], F32, tag="offx", name="offx")
            offy = poff.tile([128, QC, 64], F32, tag="offy", name="offy")

            # =========== phase B: projections ===========
            with tc.tile_pool(name="proj", bufs=1) as proj:
                queryT = [proj.tile([128, Q], F32, tag=f"qin{i}", name=f"qin{i}")
                          for i in range(2)]
                for i in range(2):
                    nc.sync.dma_start(queryT[i][:], d_queryT[i])
                qT = [proj.tile([128, Q], F32, tag=f"qT{i}", name=f"qT{i}")
                      for i in range(2)]
                for m in range(2):
                    for n in range(Q // 512):
                        ps = psum.tile([128, 512], F32, tag="mm", name="mm")
                        for k in range(2):
                            nc.tensor.matmul(ps[:], Wq[k][:, m * 128:(m + 1) * 128],
                                             queryT[k][:, n * 512:(n + 1) * 512],
                                             start=(k == 0), stop=(k == 1))
                        nc.scalar.activation(qT[m][:, n * 512:(n + 1) * 512], ps[:],
                                             AF.Identity, bias=bq[m][:], scale=1.0)
                for c in range(QC):
                    ps = psum.tile([128, 512], F32, tag="mm", name="mm")
                    for k in range(2):
                        nc.tensor.matmul(ps[:, 0:128], qT[k][:, c * 128:(c + 1) * 128],
                                         Woff[k][:], start=(k == 0), stop=(k == 1))
                    nc.scalar.activation(offx[:, c], ps[:, 0:64], AF.Copy)
                    nc.scalar.activation(offy[:, c], ps[:, 64:128], AF.Copy)
                    ps2 = psum.tile([128, 512], F32, tag="mm", name="mm")
                    for k in range(2):
                        nc.tensor.matmul(ps2[:, 0:64], qT[k][:, c * 128:(c + 1) * 128],
                                         WA[k][:], start=(k == 0), stop=(k == 1))
                    nc.scalar.activation(Aw[:, c], ps2[:, 0:64], AF.Copy)
                nc.vector.tensor_tensor(offx[:], offx[:], bh(boffE[:, 0:64]), OP.add)
                nc.vector.tensor_tensor(offy[:], offy[:], bh(boffE[:, 64:128]), OP.add)
                nc.vector.tensor_tensor(Aw[:], Aw[:], bh(bAE), OP.add)
                nc.scalar.activation(Aw[:], Aw[:], AF.Exp)
                Aw4 = Aw[:].rearrange("p c (h s) -> p c h s", s=16)
                ssum = pbc.tile([128, QC, HL], F32, tag="ssum", name="ssum")
                nc.vector.tensor_reduce(ssum[:], Aw4, mybir.AxisListType.X, OP.add)
                nc.vector.reciprocal(ssum[:], ssum[:])
                rb = ssum[:].rearrange("p c (h o) -> p c h o", o=1) \
                            .broadcast_to([128, QC, HL, 16])
                nc.vector.tensor_tensor(Aw4, Aw4, rb, OP.mult)

            # =========== phase C: coords, weights, tokens ===========
            with tc.tile_pool(name="coord", bufs=1) as cp:
                def ct(tag, dt=F32):
                    return cp.tile([128, QC, 64], dt, tag=tag, name=tag)

                ix = ct("ix"); iy = ct("iy")
                for c in range(QC):
                    nc.vector.tensor_scalar(ix[:, c], C["wl_t"][:], refx[:, c:c + 1],
                                            -0.5, OP.mult, OP.add)
                    nc.vector.tensor_scalar(iy[:, c], C["hl_t"][:], refy[:, c:c + 1],
                                            -0.5, OP.mult, OP.add)
                nc.vector.tensor_tensor(offx[:], offx[:], bh(C["cofx_t"]), OP.mult)
                nc.vector.tensor_tensor(offy[:], offy[:], bh(C["cofy_t"]), OP.mult)
                nc.vector.tensor_tensor(ix[:], ix[:], offx[:], OP.add)
                nc.vector.tensor_tensor(iy[:], iy[:], offy[:], OP.add)

                res = {}
                for dim, (iv, lim1, lim2) in enumerate(
                        ((ix, C["wlm1_t"], C["wlm2_t"]),
                         (iy, C["hlm1_t"], C["hlm2_t"]))):
                    xm = ct("xm")
                    nc.vector.tensor_scalar(xm[:], iv[:], 0.5, None, OP.subtract)
                    xi = ct("xi", I32)
                    nc.vector.tensor_copy(xi[:], xm[:])      # RNE => floor(iv)
                    x0 = ct(f"x0_{dim}")
                    nc.vector.tensor_copy(x0[:], xi[:])
                    w1 = ct(f"w1_{dim}")
                    nc.vector.tensor_tensor(w1[:], iv[:], x0[:], OP.subtract)
                    w0 = ct(f"w0_{dim}")
                    nc.vector.tensor_scalar(w0[:], w1[:], -1.0, 1.0, OP.mult, OP.add)
                    m = ct("mA")
                    mb = ct("mB")
                    nc.vector.tensor_scalar(m[:], x0[:], 0.0, None, OP.is_ge)
                    nc.vector.tensor_tensor(mb[:], x0[:], bh(lim1), OP.is_le)
                    nc.vector.tensor_tensor(m[:], m[:], mb[:], OP.mult)
                    nc.vector.tensor_tensor(w0[:], w0[:], m[:], OP.mult)
                    nc.vector.tensor_scalar(m[:], x0[:], -1.0, None, OP.is_ge)
                    nc.vector.tensor_tensor(mb[:], x0[:], bh(lim2), OP.is_le)
                    nc.vector.tensor_tensor(m[:], m[:], mb[:], OP.mult)
                    nc.vector.tensor_tensor(w1[:], w1[:], m[:], OP.mult)
                    res[dim] = (x0, w0, w1)
                x0f, wx0, wx1 = res[0]
                y0f, wy0, wy1 = res[1]

                W4v = W4[:].rearrange("p h s k c f d -> p c (h s k) f d")
                u = ct("mA")
                for cy, wyv in ((0, wy0), (1, wy1)):
                    nc.vector.tensor_tensor(u[:], Aw[:], wyv[:], OP.mult)
                    for cx, wxv in ((0, wx0), (1, wx1)):
                        for dup in range(2):
                            nc.vector.tensor_tensor(
                                W4v[:, :, :, cy * 2 + cx, dup],
                                u[:], wxv[:], OP.mult)

                nc.vector.tensor_scalar(y0f[:], y0f[:], -1.0, None, OP.max)
                nc.vector.tensor_tensor(y0f[:], y0f[:], bh(C["hlm1_t"]), OP.min)
                nc.vector.tensor_scalar(x0f[:], x0f[:], -1.0, None, OP.max)
                nc.vector.tensor_tensor(x0f[:], x0f[:], bh(C["wl_t"]), OP.min)
                tokf = ct("xm")
                nc.vector.tensor_tensor(tokf[:], y0f[:], bh(C["wl_t"]), OP.mult)
                nc.vector.tensor_tensor(tokf[:], tokf[:], x0f[:], OP.add)
                nc.vector.tensor_tensor(tokf[:], tokf[:], bh(C["wlp1_t"]), OP.add)
                toki = ct("xi", I32)
                nc.vector.tensor_copy(toki[:], tokf[:])
                nc.vector.tensor_copy(tok16[:], toki[:].bitcast(I16)[:, :, 0:128:2])

        # wrapped gather-index layout, chunk = (qh, hl, s):
        # IDXW[p%16, qh, hl, s, k, qc, p//16] = tok16[p, qh*16+qc, (hl,s,k)]
        idxp = ctx.enter_context(tc.tile_pool(name="idxp", bufs=1))
        IDXW = idxp.tile([128, 2, HL, SCALES, KPTS, 16, 8], I16,
                         tag="IDXW", name="IDXW")
        for ph in range(8 if "I" in PH else 0):
            for qcg in range(QC):
                qh, qcl = qcg // 16, qcg % 16
                eng = nc.sync if (ph + qcg) % 2 == 0 else nc.scalar
                eng.dma_start(IDXW[0:16, qh, :, :, :, qcl, ph]
                              .rearrange("p h s k -> p (h s k)"),
                              tok16[ph * 16:(ph + 1) * 16, qcg, :])
        for d0, n in ((16, 16), (32, 32), (64, 64)):
            nc.sync.dma_start(IDXW[d0:d0 + n], IDXW[0:n])

        # =========== phase D: quad maps ===========
        if "D" in PH:
          with tc.tile_pool(name="mapp", bufs=3) as mp:
            for l in range(SCALES):
                hl_, wl = HW_SIZES[l]
                dmv = d_map[l][:].rearrange("h t e -> h (t e)")
                nslab = max(POS[l] // 512, 1)
                slab = POS[l] // nslab
                SUP = min(4, nslab)          # matmul slabs per slot-write
                stg = None
                for sl in range(nslab):
                    if sl % SUP == 0:
                        stg = mp.tile([128, 4 * SUP, 128], BF16, tag="stg",
                                      name="stg")
                    kin = [mp.tile([128, slab], F32, tag=f"kin{i}", name=f"kin{i}")
                           for i in range(2)]
                    for i in range(2):
                        nc.sync.dma_start(kin[i][:],
                                          d_keysT[l][i, :, sl * slab:(sl + 1) * slab])
                    kfs = mp.tile([128, slab], F32, tag="kfs", name="kfs")
                    ps = psum.tile([128, 512], F32, tag="mm", name="mm")
                    for k in range(2):
                        nc.tensor.matmul(ps[:, 0:slab], Wk[k][:],
                                         kin[k][:], start=(k == 0), stop=(k == 1))
                    nc.scalar.activation(kfs[:], ps[:, 0:slab], AF.Identity,
                                         bias=bk[:], scale=1.0)
                    nsub = slab // 128
                    for sub in range(nsub):
                        pt = psum.tile([128, 128], F32, tag="tp", name="tp")
                        nc.tensor.transpose(
                            pt[:], kfs[:, sub * 128:(sub + 1) * 128], ident[:])
                        nc.scalar.activation(stg[:, (sl % SUP) * nsub + sub], pt[:],
                                             AF.Copy)
                    if sl % SUP == SUP - 1:
                        sup0 = (sl // SUP) * SUP
                        supsz = SUP * slab
                        src_ap = stg[:, 0:SUP * nsub] \
                            .rearrange("p a (hl c) -> p hl a c", c=DK)
                        for cy in range(2):
                            for cx in range(2):
                                base = sup0 * slab + wl + 1 - cy * wl - cx
                                start = base * 128 + (cy * 2 + cx) * DK
                                dst4 = dmv[:, start:start + supsz * 128] \
                                    .rearrange("h (t e) -> h t e", e=128)[:, :, 0:DK] \
                                    .rearrange("h (a p) e -> p h a e", p=128)
                                for hh in range(HL):
                                    eng = nc.sync if hh % 2 == 0 else nc.scalar
                                    eng.dma_start(dst4[:, hh], src_ap[:, hh])

        tc.strict_bb_all_engine_barrier()

        # =========== phase E: gather + interpolate ===========
        nc.vector.memset(feat[:], 0)
        if "E" in PH:
          with tc.tile_pool(name="gath", bufs=1) as gp:
            for h in range(HL):
                for l in range(SCALES):
                    for qh in range(2):
                        G = gp.tile([128, KPTS * 16, 128], BF16, tag="G", name="G",
                                    bufs=2)
                        nc.gpsimd.dma_gather(
                            G[:], d_map[l][h],
                            IDXW[:, qh, h, l].rearrange("p k c e -> p (k c e)"),
                            num_idxs=NIDX, num_idxs_reg=NIDX,
                            elem_size=128, elem_step=128, single_packet=False)
                        M = gp.tile([128, KPTS * 16, 128], BF16, tag="M", name="M")
                        for kk in range(KPTS):
                            wv = W4[:, h, l, kk, qh * 16:(qh + 1) * 16, :, :] \
                                .rearrange("p c f (o d) -> p c f o d", o=1) \
                                .broadcast_to([128, 16, 4, 16, 2])
                            nc.vector.tensor_tensor(
                                M[:, kk * 16:(kk + 1) * 16]
                                    .rearrange("p c (f a b) -> p c f a b", a=16, b=2),
                                G[:, kk * 16:(kk + 1) * 16]
                                    .rearrange("p c (f a b) -> p c f a b", a=16, b=2),
                                wv, OP.mult)
                        r1 = gp.tile([128, KPTS * 16, 64], BF16, tag="r1", name="r1")
                        nc.vector.tensor_tensor(r1[:], M[:, :, 0:64], M[:, :, 64:128],
                                                OP.add)
                        r2 = gp.tile([128, KPTS * 16, DK], BF16, tag="r2", name="r2")
                        nc.vector.tensor_tensor(r2[:], r1[:, :, 0:32], r1[:, :, 32:64],
                                                OP.add)
                        t1 = gp.tile([128, 2 * 16, DK], BF16, tag="t1", name="t1")
                        nc.vector.tensor_tensor(t1[:], r2[:, 0:32], r2[:, 32:64],
                                                OP.add)
                        t2 = gp.tile([128, 16, DK], F32, tag="t2", name="t2")
                        nc.vector.tensor_tensor(t2[:], t1[:, 0:16], t1[:, 16:32],
                                                OP.add)
                        fslice = feat[:, qh * 16:(qh + 1) * 16, h]
                        nc.vector.tensor_tensor(fslice, fslice, t2[:], OP.add)

        # =========== phase F: scramble via DRAM + output projection ===========
        # featD[o, c'] = feat[q=8t+e, h, d] with o = h*512 + t, c' = e*32 + d
        fD = d_featD[:].rearrange("o c -> (o c)")
        for e in range(8):
            for h in range(HL):
                src = feat[e:128:8, :, h, :]
                dst = bass.AP(fD.tensor,
                              fD.offset + h * 512 * 256 + e * DK,
                              ((256, 16), (16 * 256, QC), (1, DK)))
                eng = nc.sync if (e + h) % 2 == 0 else nc.scalar
                eng.dma_start(dst, src)

        tc.strict_bb_all_engine_barrier()

        with tc.tile_pool(name="outp", bufs=1) as op:
            INq = op.tile([128, 16, 256], F32, tag="INq", name="INq")
            nc.sync.dma_start(
                INq[:], d_featD[:].rearrange("(a p) c -> p a c", p=128))
            featT = [op.tile([128, 2048], F32, tag=f"fT{i}", name=f"fT{i}")
                     for i in range(2)]
            for ch in range(16):
                for m in range(2):
                    pt = psum.tile([128, 128], F32, tag="tp", name="tp")
                    nc.tensor.transpose(pt[:], INq[:, ch, m * 128:(m + 1) * 128],
                                        ident[:])
                    nc.scalar.activation(featT[m][:, ch * 128:(ch + 1) * 128], pt[:],
                                         AF.Copy)
            outT = [op.tile([128, 2048], F32, tag=f"oT{i}", name=f"oT{i}")
                    for i in range(2)]
            for m in range(2):
                for n in range(2048 // 512):
                    ps = psum.tile([128, 512], F32, tag="mm", name="mm")
                    for k in range(2):
                        nc.tensor.matmul(ps[:], Wm[k][:, m * 128:(m + 1) * 128],
                                         featT[k][:, n * 512:(n + 1) * 512],
                                         start=(k == 0), stop=(k == 1))
                    nc.scalar.activation(outT[m][:, n * 512:(n + 1) * 512], ps[:],
                                         AF.Identity, bias=bm[m][:], scale=1.0)
                nc.sync.dma_start(d_out[m], outT[m][:])

    nc.compile()
    return nc


def _prep_inputs(query, keys, ref_point, Wq, bq, Wk, bk, Woff, boff, WA, bA, Wm, bm):
    def two(w, n):
        return np.ascontiguousarray(w.reshape(2, 128, n).astype(np.float32))

    wl_arr = np.zeros(64, np.float32)
    hl_arr = np.zeros(64, np.float32)
    for h in range(HL):
        for s in range(SCALES):
            for k in range(KPTS):
                hl_, wl_ = HW_SIZES[s]
                wl_arr[h * 16 + s * 4 + k] = wl_
                hl_arr[h * 16 + s * 4 + k] = hl_
    consts = {
        "wl_t": np.tile(wl_arr, (128, 1)),
        "wlm1_t": np.tile(wl_arr - 1, (128, 1)),
        "wlm2_t": np.tile(wl_arr - 2, (128, 1)),
        "wlp1_t": np.tile(wl_arr + 1, (128, 1)),
        "hlm1_t": np.tile(hl_arr - 1, (128, 1)),
        "hlm2_t": np.tile(hl_arr - 2, (128, 1)),
        "hl_t": np.tile(hl_arr, (128, 1)),
        "cofx_t": np.tile(wl_arr / (wl_arr - 1), (128, 1)),
        "cofy_t": np.tile(hl_arr / (hl_arr - 1), (128, 1)),
    }
    consts = {k: np.ascontiguousarray(v.astype(np.float32)) for k, v in consts.items()}

    rs = ref_point.reshape(Q, 2)
    refx = np.ascontiguousarray(rs[:, 0].reshape(QC, 128).T)
    refy = np.ascontiguousarray(rs[:, 1].reshape(QC, 128).T)

    in_maps = []
    for core in range(NCORES):
        b, hg = core // 2, core % 2
        heads = range(4 * hg, 4 * hg + 4)
        perm_off = np.zeros(128, np.int64)
        perm_A = np.zeros(64, np.int64)
        for i, h in enumerate(heads):
            for s in range(SCALES):
                for k in range(KPTS):
                    for xy in range(2):
                        perm_off[xy * 64 + i * 16 + s * 4 + k] = \
                            ((h * SCALES + s) * KPTS + k) * 2 + xy
                    perm_A[i * 16 + s * 4 + k] = (h * SCALES + s) * KPTS + k
        WoffP = np.ascontiguousarray(Woff[:, perm_off])
        boffP = boff[perm_off]
        WAP = np.ascontiguousarray(WA[:, perm_A])
        bAP = bA[perm_A]
        chs = slice(4 * hg * DK, (4 * hg + 4) * DK)
        m = {
            "Wq": two(Wq, D), "Wk": two(np.ascontiguousarray(Wk[:, chs]), 128),
            "WoffP": two(WoffP, 128), "WA": two(WAP, 64), "Wm": two(Wm, D),
            "bq": two(bq, 1), "bm": two(bm, 1),
            "bk": np.ascontiguousarray(bk[chs]).reshape(128, 1).astype(np.float32),
            "boffE": np.tile(boffP, (128, 1)).astype(np.float32),
            "bAE": np.tile(bAP, (128, 1)).astype(np.float32),
            "refx": refx, "refy": refy, **consts,
        }
        qs = query[b].reshape(Q, D)
        m["queryT"] = np.ascontiguousarray(qs.T).reshape(2, 128, Q)
        for l in range(SCALES):
            m[f"keysT{l}"] = np.ascontiguousarray(
                keys[l][b].reshape(POS[l], D).T).reshape(2, 128, POS[l])
        in_maps.append(m)
    return in_maps


def kernel(query, keys0, keys1, keys2, keys3, ref_point,
           Wq, bq, Wk, bk, Woff, boff, WA, bA, Wm, bm):
    query = np.asarray(query, np.float32)
    keys = [np.asarray(k, np.float32) for k in (keys0, keys1, keys2, keys3)]
    in_maps = _prep_inputs(
        query, keys, np.asarray(ref_point, np.float32),
        np.asarray(Wq, np.float32), np.asarray(bq, np.float32),
        np.asarray(Wk, np.float32), np.asarray(bk, np.float32),
        np.asarray(Woff, np.float32), np.asarray(boff, np.float32),
        np.asarray(WA, np.float32), np.asarray(bA, np.float32),
        np.asarray(Wm, np.float32), np.asarray(bm, np.float32))
    if "nc" not in _cache:
        _cache["nc"] = _build()
    nc = _cache["nc"]
    res = run_bass_kernel_spmd(nc, in_maps, list(range(NCORES)))
    out = np.zeros((B, H, W, D), np.float32)
    for core in range(NCORES):
        b, hg = core // 2, core % 2
        oT = res.results[core]["outT"].reshape(D, 2048)
        out[b, 32 * hg:32 * hg + 32] = oT.T.reshape(32, W, D)
    return out



# revision 27
# speedup vs baseline: 1.2596x; 1.2596x over previous
"""Deformable head attention kernel for 8 Trainium2 NeuronCores.

Sharding: core i handles batch b = i//2 and head-group hg = i%2 (heads
4*hg..4*hg+3, all 4096 queries). The reference's final reshape maps output
pixel p' to head p'//512's features of queries 8t..8t+7 (t = p' % 512), so a
head-group owns output rows [hg*32, hg*32+32) exactly -- fully local per core.

Per core (v2, overlap-oriented):
  - channel-major matmuls on PE (f32r fast path where exact values allow)
  - per-(head,scale) quad maps (2x2x32ch tokens, 256B bf16) in DRAM, written
    with merged 4-head descriptors; pad regions zero-filled byte-disjointly
  - gather indices built with PE "selection matmuls" (ident slices) instead
    of 256 small DMAs: partitions ph*16+r fold to idx partition r with ph in
    the free dim, exactly the SWDGE 16-wrap layout
  - coordinate/weight math split x-on-DVE / y-on-Pool
  - SWDGE dma_gather fetches one quad per sample; weighted reduce split
    DVE (mult+first add) / Pool (tail adds) so the gather train is DMA-bound
  - per-head output scramble via DRAM tile + Wm projection (pipelined tail)
"""
import numpy as np
import ml_dtypes
from contextlib import ExitStack

BFNP = ml_dtypes.bfloat16

import concourse.bass as bass
import concourse.tile as tile
from concourse import bacc, mybir
from concourse.bass_utils import run_bass_kernel_spmd
from concourse.masks import make_identity

F32 = mybir.dt.float32
F32R = mybir.dt.float32r
I32 = mybir.dt.int32
I16 = mybir.dt.int16
BF16 = mybir.dt.bfloat16
OP = mybir.AluOpType
AF = mybir.ActivationFunctionType

HEADS, KPTS, SCALES, D = 8, 4, 4, 256
DK = D // HEADS              # 32
HL = 4                       # heads per core
B, H, W = 4, 64, 64
Q = 4096                     # queries per core (full image)
QC = Q // 128                # 32 q-chunks
HW_SIZES = [(16, 16), (32, 32), (64, 64), (128, 128)]
POS = [h * w for h, w in HW_SIZES]
TCAP = [p + w + 4 for p, (h, w) in zip(POS, HW_SIZES)]
NCORES = 8
NIDX = KPTS * 2048           # 8192 gather indices per (head, scale, q-half)
LS = 2048                    # key-projection slab (tokens)

_cache = {}


def _build():
    nc = bacc.Bacc("TRN2", target_bir_lowering=False, debug=False)

    d_queryT = nc.dram_tensor("queryT", [2, 128, Q], BF16, kind="ExternalInput")
    d_keysT = [nc.dram_tensor(f"keysT{l}", [2, 128, POS[l]], BF16,
                              kind="ExternalInput")
               for l in range(SCALES)]
    d_refx = nc.dram_tensor("refx", [128, QC], F32, kind="ExternalInput")
    d_refy = nc.dram_tensor("refy", [128, QC], F32, kind="ExternalInput")
    d_Wq = nc.dram_tensor("Wq", [2, 128, D], BF16, kind="ExternalInput")
    d_Wk = nc.dram_tensor("Wk", [2, 128, 128], BF16, kind="ExternalInput")
    d_Woff = nc.dram_tensor("WoffP", [2, 128, 128], BF16, kind="ExternalInput")
    d_WA = nc.dram_tensor("WA", [2, 128, 64], BF16, kind="ExternalInput")
    d_Wm = nc.dram_tensor("Wm", [2, 128, D], F32, kind="ExternalInput")
    d_bq = nc.dram_tensor("bq", [2, 128, 1], F32, kind="ExternalInput")
    d_bk = nc.dram_tensor("bk", [128, 1], F32, kind="ExternalInput")
    d_bm = nc.dram_tensor("bm", [2, 128, 1], F32, kind="ExternalInput")
    d_boff = nc.dram_tensor("boffE", [128, 128], F32, kind="ExternalInput")
    d_bA = nc.dram_tensor("bAE", [128, 64], F32, kind="ExternalInput")
    cnames = ["wl_t", "wlm1_t", "wlm2_t", "wlp1_t", "hlm1_t", "hlm2_t",
              "hl_t", "cofx_t", "cofy_t"]
    d_const = {n: nc.dram_tensor(n, [128, 64], F32, kind="ExternalInput")
               for n in cnames}

    d_out = nc.dram_tensor("outT", [2, 128, 2048], F32, kind="ExternalOutput")

    def r32(ap):
        return ap.bitcast(F32R)

    with tile.TileContext(nc) as tc, ExitStack() as ctx:
        wpool = ctx.enter_context(tc.tile_pool(name="weights", bufs=1))
        ppool = ctx.enter_context(tc.tile_pool(name="persist", bufs=1))
        psum = ctx.enter_context(tc.tile_pool(name="psum", bufs=2, space="PSUM"))
        psumb = ctx.enter_context(tc.tile_pool(name="psumb", bufs=2, space="PSUM"))
        dram = ctx.enter_context(tc.tile_pool(name="dramp", bufs=1, space="DRAM"))

        d_map = [dram.tile([HL, TCAP[l], 128], BF16, tag=f"map{l}", name=f"map{l}")
                 for l in range(SCALES)]
        d_featD = dram.tile([2048, 256], F32, tag="featD", name="featD")

        def load2(d, n, nm, dt=F32):
            t = [wpool.tile([128, n], dt, tag=f"{nm}{i}", name=f"{nm}{i}")
                 for i in range(2)]
            for i in range(2):
                nc.sync.dma_start(t[i][:], d[i])
            return t

        def load1(d, shape, nm):
            t = wpool.tile(shape, F32, tag=nm, name=nm)
            nc.sync.dma_start(t[:], d[:])
            return t

        Wq = load2(d_Wq, D, "Wq", BF16); Wk = load2(d_Wk, 128, "Wk", BF16)
        Woff = load2(d_Woff, 128, "Woff", BF16)
        WA = load2(d_WA, 64, "WA", BF16)
        Wm = load2(d_Wm, D, "Wm")
        bq = load2(d_bq, 1, "bq"); bm = load2(d_bm, 1, "bm")
        bk = load1(d_bk, [128, 1], "bk")
        boffE = load1(d_boff, [128, 128], "boffE")
        bAE = load1(d_bA, [128, 64], "bAE")
        C = {n: load1(d_const[n], [128, 64], n) for n in cnames}
        refx = load1(d_refx, [128, QC], "refx")
        refy = load1(d_refy, [128, QC], "refy")
        identF = wpool.tile([128, 128], F32, tag="identF", name="identF")
        make_identity(nc, identF[:])
        identB = wpool.tile([128, 128], BF16, tag="identB", name="identB")
        make_identity(nc, identB[:])
        zt = wpool.tile([128, HL, 32], BF16, tag="zt", name="zt")
        nc.vector.memset(zt[:], 0)

        # persistent state
        W4 = ppool.tile([128, HL, SCALES, KPTS, QC, 4, 2], BF16, tag="W4",
                        name="W4")
        feat = ppool.tile([128, QC, HL, DK], F32, tag="feat", name="feat")
        IDX16 = ppool.tile([128, 2, HL, SCALES, KPTS, 16, 8], I16,
                           tag="IDX16", name="IDX16")
        nc.vector.memset(IDX16[:], 0)
        nc.vector.memset(feat[:], 0)

        # =========== quad map construction (phase D) ===========
        mp = tc.alloc_tile_pool(name="mapp", bufs=1)
        CORNERS = [(0, 0), (0, 1), (1, 0), (1, 1)]

        def zero_edges(l):
            """Fill every map byte not covered by a corner write with zeros.
            Corner (cy,cx) covers entries [base, base+POS) at entry-offset
            (cy*2+cx)*32; zero the complement so reads never see garbage."""
            mapl = d_map[l]
            hl_, wl = HW_SIZES[l]
            for cy, cx in CORNERS:
                base = wl + 1 - cy * wl - cx
                off = (cy * 2 + cx) * 32
                for t0, t1 in ((0, base), (base + POS[l], TCAP[l])):
                    t = t0
                    while t < t1:
                        n = min(128, t1 - t)
                        dst = bass.AP(
                            mapl.tensor,
                            mapl.offset + t * 128 + off,
                            ((128, n), (TCAP[l] * 128, HL), (1, 32)))
                        nc.sync.dma_start(dst, zt[0:n, :, :])
                        t += n

        def build_map(l):
            zero_edges(l)
            hl_, wl = HW_SIZES[l]
            mapl = d_map[l]
            slab = min(LS, POS[l])
            nslab = POS[l] // slab
            for sl in range(nslab):
                kin = [mp.tile([128, slab], BF16, tag=f"kin{i}", name=f"kin{i}",
                               bufs=2) for i in range(2)]
                for i in range(2):
                    nc.sync.dma_start(kin[i][:],
                                      d_keysT[l][i, :, sl * slab:(sl + 1) * slab])
                kfs = mp.tile([128, slab], BF16, tag="kfs", name="kfs", bufs=2)
                for c0 in range(0, slab, 512):
                    cw = min(512, slab - c0)
                    ps = psum.tile([128, 512], F32, tag="mm", name="mm")
                    for k in range(2):
                        nc.tensor.matmul(ps[:, 0:cw], Wk[k][:],
                                         kin[k][:, c0:c0 + cw],
                                         start=(k == 0), stop=(k == 1))
                    nc.scalar.activation(kfs[:, c0:c0 + cw], ps[:, 0:cw],
                                         AF.Identity, bias=bk[:], scale=1.0)
                nsub = slab // 128
                stg = mp.tile([128, nsub, 128], BF16, tag="stg", name="stg",
                              bufs=2)
                for g0 in range(0, nsub, 4):
                    gw = min(4, nsub - g0)
                    pt = psumb.tile([128, 512], BF16, tag="tp", name="tp")
                    for j in range(gw):
                        nc.tensor.transpose(
                            pt[:, j * 128:(j + 1) * 128],
                            kfs[:, (g0 + j) * 128:(g0 + j + 1) * 128],
                            identB[:])
                    nc.scalar.activation(
                        stg[:, g0:g0 + gw].rearrange("p a e -> p (a e)"),
                        pt[:, 0:gw * 128], AF.Copy)
                src = stg[:].rearrange("p a (h e) -> p h a e", e=DK)
                for cy, cx in CORNERS:
                    base = wl + 1 - cy * wl - cx
                    off = (base + sl * slab) * 128 + (cy * 2 + cx) * DK
                    for hh in range(HL):
                        dst = bass.AP(
                            mapl.tensor,
                            mapl.offset + off + hh * TCAP[l] * 128,
                            ((128, 128), (128 * 128, nsub), (1, DK)))
                        eng = nc.sync if (cy * 2 + cx + hh) % 2 == 0 else nc.scalar
                        eng.dma_start(dst, src[:, hh])

        for l in range(3):
            build_map(l)

        # =========== phase B: q / offset / attention projections ===========
        pbc = tc.alloc_tile_pool(name="pbc", bufs=1)
        Aw = pbc.tile([128, QC, 64], F32, tag="Aw", name="Aw")
        offx = pbc.tile([128, QC, 64], F32, tag="offx", name="offx")
        offy = pbc.tile([128, QC, 64], F32, tag="offy", name="offy")

        with tc.tile_pool(name="proj", bufs=1) as proj:
            queryT = [proj.tile([128, Q], BF16, tag=f"qin{i}", name=f"qin{i}")
                      for i in range(2)]
            for i in range(2):
                nc.sync.dma_start(queryT[i][:], d_queryT[i])
            qT = [proj.tile([128, Q], BF16, tag=f"qT{i}", name=f"qT{i}")
                  for i in range(2)]
            for m in range(2):
                for n in range(Q // 512):
                    ps = psum.tile([128, 512], F32, tag="mm", name="mm")
                    for k in range(2):
                        nc.tensor.matmul(ps[:], Wq[k][:, m * 128:(m + 1) * 128],
                                         queryT[k][:, n * 512:(n + 1) * 512],
                                         start=(k == 0), stop=(k == 1))
                    nc.scalar.activation(qT[m][:, n * 512:(n + 1) * 512], ps[:],
                                         AF.Identity, bias=bq[m][:], scale=1.0)
            for c in range(QC):
                ps = psum.tile([128, 512], F32, tag="mm", name="mm")
                for k in range(2):
                    nc.tensor.matmul(ps[:, 0:128], qT[k][:, c * 128:(c + 1) * 128],
                                     Woff[k][:], start=(k == 0), stop=(k == 1))
                nc.scalar.activation(offx[:, c], ps[:, 0:64], AF.Copy)
                nc.scalar.activation(offy[:, c], ps[:, 64:128], AF.Copy)
                ps2 = psum.tile([128, 512], F32, tag="mm", name="mm")
                for k in range(2):
                    nc.tensor.matmul(ps2[:, 0:64], qT[k][:, c * 128:(c + 1) * 128],
                                     WA[k][:], start=(k == 0), stop=(k == 1))
                nc.scalar.activation(Aw[:, c], ps2[:, 0:64], AF.Copy)

        # =========== phase C: coords, weights, tokens (x: DVE, y: Pool) ====
        tokp = tc.alloc_tile_pool(name="tokp", bufs=1)
        tokf = [tokp.tile([128, 16, 64], F32, tag=f"tokf{qh}", name=f"tokf{qh}")
                for qh in range(2)]
        NQ = 8                   # q-chunks per coord quarter

        def bhq(t, n=64):
            return t[:].rearrange("p (o f) -> p o f", o=1).broadcast_to([128, NQ, n])

        for qq in range(QC // NQ):
            sl = slice(qq * NQ, (qq + 1) * NQ)
            with tc.tile_pool(name=f"coord{qq}", bufs=1) as cp:
                def ct(tag, dt=F32):
                    return cp.tile([128, NQ, 64], dt, tag=tag, name=tag)

                ix = ct("ix"); iy = ct("iy")
                for c in range(NQ):
                    cc = qq * NQ + c
                    nc.vector.tensor_scalar(ix[:, c], C["wl_t"][:],
                                            refx[:, cc:cc + 1], -0.5,
                                            OP.mult, OP.add)
                    nc.gpsimd.tensor_scalar(iy[:, c], C["hl_t"][:],
                                            refy[:, cc:cc + 1], -0.5,
                                            OP.mult, OP.add)
                oxv = offx[:, sl]
                oyv = offy[:, sl]
                nc.vector.tensor_tensor(oxv, oxv, bhq(C["cofx_t"]), OP.mult)
                nc.gpsimd.tensor_tensor(oyv, oyv, bhq(C["cofy_t"]), OP.mult)
                nc.vector.tensor_tensor(ix[:], ix[:], oxv, OP.add)
                nc.gpsimd.tensor_tensor(iy[:], iy[:], oyv, OP.add)

                def dimchain(eng, iv, lim1, lim2, pre):
                    xm = ct(f"xm{pre}")
                    eng.tensor_scalar(xm[:], iv[:], 0.5, None, OP.subtract)
                    xi = ct(f"xi{pre}", I32)
                    eng.tensor_copy(xi[:], xm[:])      # RNE => floor(iv)
                    x0 = ct(f"x0{pre}")
                    eng.tensor_copy(x0[:], xi[:])
                    w1 = ct(f"w1{pre}")
                    eng.tensor_tensor(w1[:], iv[:], x0[:], OP.subtract)
                    w0 = ct(f"w0{pre}")
                    eng.tensor_scalar(w0[:], w1[:], -1.0, 1.0, OP.mult, OP.add)
                    m = ct(f"m{pre}")
                    mb = ct(f"mb{pre}")
                    eng.tensor_scalar(m[:], x0[:], 0.0, None, OP.is_ge)
                    eng.tensor_tensor(mb[:], x0[:], bhq(lim1), OP.is_le)
                    eng.tensor_tensor(m[:], m[:], mb[:], OP.mult)
                    eng.tensor_tensor(w0[:], w0[:], m[:], OP.mult)
                    eng.tensor_scalar(m[:], x0[:], -1.0, None, OP.is_ge)
                    eng.tensor_tensor(mb[:], x0[:], bhq(lim2), OP.is_le)
                    eng.tensor_tensor(m[:], m[:], mb[:], OP.mult)
                    eng.tensor_tensor(w1[:], w1[:], m[:], OP.mult)
                    return x0, w0, w1

                x0f, wx0, wx1 = dimchain(nc.vector, ix, C["wlm1_t"],
                                         C["wlm2_t"], "x")
                y0f, wy0, wy1 = dimchain(nc.gpsimd, iy, C["hlm1_t"],
                                         C["hlm2_t"], "y")

                # softmax over (s, k) per head for this q-quarter
                Av = Aw[:, sl]
                nc.vector.tensor_tensor(Av, Av, bhq(bAE), OP.add)
                nc.scalar.activation(Av, Av, AF.Exp)
                Aw4 = Av.rearrange("p c (h s) -> p c h s", s=16)
                ssum = cp.tile([128, NQ, HL], F32, tag="ssum", name="ssum")
                nc.vector.tensor_reduce(ssum[:], Aw4, mybir.AxisListType.X,
                                        OP.add)
                nc.vector.reciprocal(ssum[:], ssum[:])
                rb = ssum[:].rearrange("p c (h o) -> p c h o", o=1) \
                            .broadcast_to([128, NQ, HL, 16])
                nc.vector.tensor_tensor(Aw4, Aw4, rb, OP.mult)

                # combined interp weights
                W4v = W4[:].rearrange("p h s k c f d -> p c (h s k) f d")[:, sl]
                u = ct("mx")      # reuse
                for cy, wyv in ((0, wy0), (1, wy1)):
                    nc.vector.tensor_tensor(u[:], Av, wyv[:], OP.mult)
                    for cx, wxv in ((0, wx0), (1, wx1)):
                        for dup in range(2):
                            eng = nc.vector if (cx + dup) % 2 == 0 else nc.gpsimd
                            eng.tensor_tensor(
                                W4v[:, :, :, cy * 2 + cx, dup],
                                u[:], wxv[:], OP.mult)

                # token index (+wl+1 offset baked in)
                nc.gpsimd.tensor_scalar(y0f[:], y0f[:], -1.0, None, OP.max)
                nc.gpsimd.tensor_tensor(y0f[:], y0f[:], bhq(C["hlm1_t"]),
                                        OP.min)
                nc.vector.tensor_scalar(x0f[:], x0f[:], -1.0, None, OP.max)
                nc.vector.tensor_tensor(x0f[:], x0f[:], bhq(C["wl_t"]), OP.min)
                tk = tokf[qq // 2][:, (qq % 2) * NQ:(qq % 2 + 1) * NQ]
                nc.gpsimd.tensor_tensor(tk, y0f[:], bhq(C["wl_t"]), OP.mult)
                nc.gpsimd.tensor_tensor(tk, tk, x0f[:], OP.add)
                nc.gpsimd.tensor_tensor(tk, tk, bhq(C["wlp1_t"]), OP.add)

        build_map(3)

        # =========== gather-index build: PE partition fold ===========
        # idx[r, (k, qcl, ph)] = token[p = ph*16 + r]; selection matmul with
        # an identity column slice folds partitions exactly into the SWDGE
        # 16-wrap layout. f32 matmul is exact for these integer magnitudes.
        with tc.tile_pool(name="idxb", bufs=1) as ib:
            for ph in range(8):
                for qh in range(2):
                    tsrc = tokf[qh][:].rearrange("p c h -> p (c h)")
                    for ck in range(2):
                        ps = psum.tile([128, 512], F32, tag="mm", name="mm")
                        nc.tensor.matmul(ps[0:16, :],
                                         identF[:, ph * 16:ph * 16 + 16],
                                         tsrc[:, ck * 512:(ck + 1) * 512],
                                         start=True, stop=True)
                        stf = ib.tile([128, 512], F32, tag="stf", name="stf",
                                      bufs=2)
                        nc.scalar.activation(stf[0:16, :], ps[0:16, :], AF.Copy)
                        sti = ib.tile([128, 512], I32, tag="sti", name="sti",
                                      bufs=2)
                        nc.vector.tensor_copy(sti[0:16, :], stf[0:16, :])
                        dst = IDX16[0:16, qh, :, :, :, ck * 8:(ck + 1) * 8, ph]
                        src = sti[0:16].bitcast(I16)[:, 0:1024:2] \
                            .rearrange("p (c h s k) -> p h s k c",
                                       c=8, h=HL, s=SCALES)
                        nc.gpsimd.tensor_copy(dst, src)
        tokp.release()
        pbc.release()
        mp.release()

        # =========== phase E: gather + interpolate, per-head output =======
        op = ctx.enter_context(tc.tile_pool(name="outp", bufs=1))
        fD = d_featD[:].rearrange("o c -> (o c)")
        with tc.tile_pool(name="gath", bufs=1) as gp:
            for h in range(HL):
                for l in range(SCALES):
                    for qh in range(2):
                        G = gp.tile([128, KPTS * 16, 128], BF16, tag="G",
                                    name="G", bufs=2)
                        nc.gpsimd.dma_gather(
                            G[:], d_map[l][h],
                            IDX16[:, qh, h, l].rearrange("p k c e -> p (k c e)"),
                            num_idxs=NIDX, num_idxs_reg=NIDX,
                            elem_size=128, elem_step=128, single_packet=False)
                        M = gp.tile([128, KPTS * 16, 128], BF16, tag="M",
                                    name="M", bufs=1)
                        for kk in range(KPTS):
                            wv = W4[:, h, l, kk, qh * 16:(qh + 1) * 16, :, :] \
                                .rearrange("p c f (o d) -> p c f o d", o=1) \
                                .broadcast_to([128, 16, 4, 16, 2])
                            nc.vector.tensor_tensor(
                                M[:, kk * 16:(kk + 1) * 16]
                                    .rearrange("p c (f a b) -> p c f a b",
                                               a=16, b=2),
                                G[:, kk * 16:(kk + 1) * 16]
                                    .rearrange("p c (f a b) -> p c f a b",
                                               a=16, b=2),
                                wv, OP.mult)
                        r1 = gp.tile([128, KPTS * 16, 64], BF16, tag="r1",
                                     name="r1", bufs=2)
                        nc.vector.tensor_tensor(r1[:], M[:, :, 0:64],
                                                M[:, :, 64:128], OP.add)
                        r2 = gp.tile([128, KPTS * 16, DK], BF16, tag="r2",
                                     name="r2", bufs=2)
                        nc.gpsimd.tensor_tensor(r2[:], r1[:, :, 0:32],
                                                r1[:, :, 32:64], OP.add)
                        t1 = gp.tile([128, 2 * 16, DK], BF16, tag="t1",
                                     name="t1", bufs=2)
                        nc.gpsimd.tensor_tensor(t1[:], r2[:, 0:32],
                                                r2[:, 32:64], OP.add)
                        t2 = gp.tile([128, 16, DK], F32, tag="t2", name="t2",
                                     bufs=2)
                        nc.gpsimd.tensor_tensor(t2[:], t1[:, 0:16],
                                                t1[:, 16:32], OP.add)
                        fslice = feat[:, qh * 16:(qh + 1) * 16, h]
                        nc.gpsimd.tensor_tensor(fslice, fslice, t2[:], OP.add)

                # head h complete: scramble via DRAM + output projection.
                # featD[o, c'] = feat[q=8t+e, h, d], o = h*512 + t, c' = e*32+d
                for e in range(8):
                    src = feat[e:128:8, :, h, :]
                    dst = bass.AP(fD.tensor,
                                  fD.offset + h * 512 * 256 + e * DK,
                                  ((256, 16), (16 * 256, QC), (1, DK)))
                    eng = nc.sync if e % 2 == 0 else nc.scalar
                    eng.dma_start(dst, src)
                INq = op.tile([128, 4, 256], F32, tag="INq", name="INq", bufs=2)
                nc.sync.dma_start(
                    INq[:], d_featD[h * 512:(h + 1) * 512, :]
                        .rearrange("(a p) c -> p a c", p=128))
                featT = [op.tile([128, 512], F32, tag=f"fT{i}", name=f"fT{i}",
                                 bufs=1) for i in range(2)]
                for m in range(2):
                    pt = psumb.tile([128, 512], F32, tag="tpf", name="tpf")
                    for ch in range(4):
                        nc.tensor.transpose(
                            pt[:, ch * 128:(ch + 1) * 128],
                            INq[:, ch, m * 128:(m + 1) * 128], identF[:])
                    nc.scalar.activation(featT[m][:], pt[:], AF.Copy)
                for m in range(2):
                    ps = psum.tile([128, 512], F32, tag="mm", name="mm")
                    for k in range(2):
                        nc.tensor.matmul(ps[:], r32(Wm[k][:, m * 128:(m + 1) * 128]),
                                         r32(featT[k][:]),
                                         start=(k == 0), stop=(k == 1))
                    outT = op.tile([128, 512], F32, tag=f"oT{m}", name=f"oT{m}",
                                   bufs=2)
                    nc.scalar.activation(outT[:], ps[:], AF.Identity,
                                         bias=bm[m][:], scale=1.0)
                    nc.sync.dma_start(d_out[m][:, h * 512:(h + 1) * 512],
                                      outT[:])

    nc.compile()
    return nc


def _prep_inputs(query, keys, ref_point, Wq, bq, Wk, bk, Woff, boff, WA, bA, Wm, bm):
    def two(w, n):
        return np.ascontiguousarray(w.reshape(2, 128, n).astype(np.float32))

    wl_arr = np.zeros(64, np.float32)
    hl_arr = np.zeros(64, np.float32)
    for h in range(HL):
        for s in range(SCALES):
            for k in range(KPTS):
                hl_, wl_ = HW_SIZES[s]
                wl_arr[h * 16 + s * 4 + k] = wl_
                hl_arr[h * 16 + s * 4 + k] = hl_
    consts = {
        "wl_t": np.tile(wl_arr, (128, 1)),
        "wlm1_t": np.tile(wl_arr - 1, (128, 1)),
        "wlm2_t": np.tile(wl_arr - 2, (128, 1)),
        "wlp1_t": np.tile(wl_arr + 1, (128, 1)),
        "hlm1_t": np.tile(hl_arr - 1, (128, 1)),
        "hlm2_t": np.tile(hl_arr - 2, (128, 1)),
        "hl_t": np.tile(hl_arr, (128, 1)),
        "cofx_t": np.tile(wl_arr / (wl_arr - 1), (128, 1)),
        "cofy_t": np.tile(hl_arr / (hl_arr - 1), (128, 1)),
    }
    consts = {k: np.ascontiguousarray(v.astype(np.float32)) for k, v in consts.items()}

    rs = ref_point.reshape(Q, 2)
    refx = np.ascontiguousarray(rs[:, 0].reshape(QC, 128).T)
    refy = np.ascontiguousarray(rs[:, 1].reshape(QC, 128).T)

    in_maps = []
    for core in range(NCORES):
        b, hg = core // 2, core % 2
        heads = range(4 * hg, 4 * hg + 4)
        perm_off = np.zeros(128, np.int64)
        perm_A = np.zeros(64, np.int64)
        for i, h in enumerate(heads):
            for s in range(SCALES):
                for k in range(KPTS):
                    for xy in range(2):
                        perm_off[xy * 64 + i * 16 + s * 4 + k] = \
                            ((h * SCALES + s) * KPTS + k) * 2 + xy
                    perm_A[i * 16 + s * 4 + k] = (h * SCALES + s) * KPTS + k
        WoffP = np.ascontiguousarray(Woff[:, perm_off])
        boffP = boff[perm_off]
        WAP = np.ascontiguousarray(WA[:, perm_A])
        bAP = bA[perm_A]
        chs = slice(4 * hg * DK, (4 * hg + 4) * DK)
        m = {
            "Wq": two(Wq, D).astype(BFNP),
            "Wk": two(np.ascontiguousarray(Wk[:, chs]), 128).astype(BFNP),
            "WoffP": two(WoffP, 128).astype(BFNP),
            "WA": two(WAP, 64).astype(BFNP), "Wm": two(Wm, D),
            "bq": two(bq, 1), "bm": two(bm, 1),
            "bk": np.ascontiguousarray(bk[chs]).reshape(128, 1).astype(np.float32),
            "boffE": np.tile(boffP, (128, 1)).astype(np.float32),
            "bAE": np.tile(bAP, (128, 1)).astype(np.float32),
            "refx": refx, "refy": refy, **consts,
        }
        qs = query[b].reshape(Q, D)
        m["queryT"] = np.ascontiguousarray(qs.T).reshape(2, 128, Q).astype(BFNP)
        for l in range(SCALES):
            m[f"keysT{l}"] = np.ascontiguousarray(
                keys[l][b].reshape(POS[l], D).T).reshape(2, 128, POS[l]) \
                .astype(BFNP)
        in_maps.append(m)
    return in_maps


def kernel(query, keys0, keys1, keys2, keys3, ref_point,
           Wq, bq, Wk, bk, Woff, boff, WA, bA, Wm, bm):
    query = np.asarray(query, np.float32)
    keys = [np.asarray(k, np.float32) for k in (keys0, keys1, keys2, keys3)]
    in_maps = _prep_inputs(
        query, keys, np.asarray(ref_point, np.float32),
        np.asarray(Wq, np.float32), np.asarray(bq, np.float32),
        np.asarray(Wk, np.float32), np.asarray(bk, np.float32),
        np.asarray(Woff, np.float32), np.asarray(boff, np.float32),
        np.asarray(WA, np.float32), np.asarray(bA, np.float32),
        np.asarray(Wm, np.float32), np.asarray(bm, np.float32))
    if "nc" not in _cache:
        _cache["nc"] = _build()
    nc = _cache["nc"]
    res = run_bass_kernel_spmd(nc, in_maps, list(range(NCORES)))
    out = np.zeros((B, H, W, D), np.float32)
    for core in range(NCORES):
        b, hg = core // 2, core % 2
        oT = res.results[core]["outT"].reshape(D, 2048)
        out[b, 32 * hg:32 * hg + 32] = oT.T.reshape(32, W, D)
    return out
